# revision 4
# baseline (speedup 1.0000x reference)
"""Differential attention kernel for Trainium2, 8 NeuronCores — v2.

Sharding: B(2) x head-groups(4) -> 8 cores; each core computes 3 heads'
differential attention for one batch element plus its partial slice of the
output projection (row-parallel over Wo). Host sums the 4 partials per batch
element and adds bo.

v2 redesign vs baseline:
  * Scores: the two branches of a head run as a row-tiled PE pair
    (K=64 tiles at tile_position (0,0)/(64,0)) -> 2x score throughput.
    Layout: q_sb/k_sb per head hold branch0 on partitions 0:64 and
    branch1 on 64:128.
  * PV: col-tiled pair (M=64 tiles at (0,0)/(0,64)) -> u bank holds
    branch0 rows 0:64, branch1 rows 64:128. Softmax denominators come from
    four M=1 ones-matmuls packed at col positions 0/32/64/96 of one bank,
    accumulated across key strips.
  * exp split: branch0 -> ScalarE exact exp; branch1 on selected strips ->
    VectorE Schraudolph (tensor_scalar fp32->int16 bits of bf16 exp) to
    relieve the ACT bottleneck.
  * Output projection contracts K=128 with Wo rows duplicated to both
    partition halves, which folds the branch1-minus-branch0 reduction and
    the per-branch normalization (t = u * r broadcast) into the matmul.
  * qk projections of heads 1,2 and the half-0 output projection are
    emitted interleaved into the attention instruction stream (one aux
    PSUM bank) so the PE has fill work while ACT/DVE run exp.
"""

import os
import sys
from contextlib import ExitStack

for _p in ("/opt/trn_rl_repo", "/root/.axon_site/_ro/trn_rl_repo"):
    if os.path.isdir(_p) and _p not in sys.path:
        sys.path.insert(0, _p)

import ml_dtypes
import numpy as np

import concourse.bass as bass
import concourse.bacc as bacc_mod
import concourse.mybir as mybir
from concourse.bass_utils import run_bass_kernel_spmd
from concourse.tile import TileContext

BF16 = ml_dtypes.bfloat16
F = mybir.dt
ALU = mybir.AluOpType

B, N, C, H, D = 2, 2048, 768, 12, 64
HPC = 3          # heads per core
NCORES = 8
NT = N // 128    # 16 key strips
QH = 1024        # q processed per (head, half)

# Schraudolph exp: bf16 bits = rint(s * 128*log2(e) + (16256 - corr))
SCH_A = 128.0 * 1.4426950408889634
SCH_B = 16256.0 - 4.0
# which strips' branch-1 exp goes to the DVE (ti % DVE_MOD in DVE_SEL)
DVE_MOD = 2
DVE_SEL = (1,)


def _copy(eng, out, in_):
    if hasattr(eng, "tensor_copy"):
        eng.tensor_copy(out, in_)
    else:
        eng.copy(out, in_)


def _proj_chunk(nc, aux, dst, w_sb, xt_sb, h, g, cpeng, name):
    """One 512-wide qk-projection chunk: dst[:, h, g*512:+512]."""
    fp32 = F.float32
    pp = aux.tile([128, 512], fp32, tag="aux", name=f"aux_{name}")
    for c in range(6):
        nc.tensor.matmul(
            pp,
            lhsT=w_sb[:, c, h * 128 : (h + 1) * 128],
            rhs=xt_sb[:, c, g * 512 : (g + 1) * 512],
            start=(c == 0),
            stop=(c == 5),
        )
    cpeng_out = dst[:, h, g * 512 : (g + 1) * 512]
    _copy(cpeng, cpeng_out, pp)


def _outproj_chunk(nc, aux, t_sb, wo_sb, osb, out, ti, o, w, cpeng, dmaeng, name):
    """Output projection for row strip ti, out cols [o, o+w): K=128 over
    duplicated-wo rows folds the two-branch sum."""
    fp32 = F.float32
    fo = aux.tile([128, 512], fp32, tag="aux", name=f"aux_{name}")
    for hh in range(HPC):
        nc.tensor.matmul(
            fo[:, 0:w],
            lhsT=t_sb[:, hh, ti * 128 : (ti + 1) * 128],
            rhs=wo_sb[:, hh, o : o + w],
            start=(hh == 0),
            stop=(hh == HPC - 1),
        )
    ot = osb.tile([128, 512], F.bfloat16, tag="osb", name=f"osb_{name}")
    _copy(cpeng, ot[:, 0:w], fo[:, 0:w])
    dmaeng.dma_start(out=out[ti * 128 : (ti + 1) * 128, o : o + w], in_=ot[:, 0:w])


def _body(nc, tc, ctx, xt, wq, wk, wv, wo, lamc, out, taps=None):
    fp32, bf16 = F.float32, F.bfloat16
    Exp = mybir.ActivationFunctionType.Exp

    singles = ctx.enter_context(tc.tile_pool(name="singles", bufs=1))
    wo_sb = singles.tile([128, HPC, C], bf16)
    lam_sb = singles.tile([128, HPC], fp32)
    ones_sb = singles.tile([128, 1], bf16)
    xt_sb = singles.tile([128, 6, N], bf16)
    wq_sb = singles.tile([128, 6, HPC * 128], bf16)
    wk_sb = singles.tile([128, 6, HPC * 128], bf16)
    wv_sb = singles.tile([128, 6, HPC * D], bf16)
    q_sb = singles.tile([128, HPC, N], bf16)
    k_sb = singles.tile([128, HPC, N], bf16)
    v_sb = singles.tile([128, NT, HPC, D], bf16)
    t_sb = singles.tile([128, HPC, N], bf16)
    r_dram = nc.dram_tensor("r_bounce", [HPC * 2 * 4, 512], F.bfloat16)

    nc.vector.memset(ones_sb, 1.0)
    nc.sync.dma_start(out=lam_sb, in_=lamc[:, :])
    for h in range(HPC):
        nc.sync.dma_start(out=wo_sb[0:64, h, :], in_=wo[h * 64 : (h + 1) * 64, :])
        nc.gpsimd.dma_start(out=wo_sb[64:128, h, :], in_=wo[h * 64 : (h + 1) * 64, :])
    xt_r = xt[:, :].rearrange("(ch p) n -> p ch n", p=128)
    wq_r = wq[:, :].rearrange("(ch p) w -> p ch w", p=128)
    wk_r = wk[:, :].rearrange("(ch p) w -> p ch w", p=128)
    wv_r = wv[:, :].rearrange("(ch p) w -> p ch w", p=128)
    for c in range(6):
        nc.sync.dma_start(out=wv_sb[:, c, :], in_=wv_r[:, c, :])
    for c in range(6):
        eng = nc.sync if c % 2 == 0 else nc.gpsimd
        eng.dma_start(out=xt_sb[:, c, :], in_=xt_r[:, c, :])
    for c in range(6):
        nc.sync.dma_start(out=wq_sb[:, c, :], in_=wq_r[:, c, :])
        nc.gpsimd.dma_start(out=wk_sb[:, c, :], in_=wk_r[:, c, :])

    # pre-warm the PE clock gate during the initial DMA wait
    with tc.tile_pool(name="warm_sb", bufs=1) as warm_sb, \
         tc.tile_pool(name="warm_ps", bufs=1, space="PSUM") as warm_ps:
        wsrc = warm_sb.tile([128, 512], bf16)
        nc.vector.memset(wsrc, 0.0)
        wt = warm_ps.tile([128, 512], fp32)
        for _ in range(24):
            nc.tensor.matmul(wt, lhsT=wsrc[:, 0:128], rhs=wsrc, start=True, stop=True)

    aux = ctx.enter_context(tc.tile_pool(name="aux", bufs=1, space="PSUM"))
    osb = ctx.enter_context(tc.tile_pool(name="osb", bufs=2))

    # ---------- v projection + head-0 qk projection (phase) ----------
    with tc.tile_pool(name="vpp", bufs=2, space="PSUM") as vpp:
        for ti in range(NT):
            vp = vpp.tile([128, HPC * D], fp32)
            for c in range(6):
                nc.tensor.matmul(
                    vp,
                    lhsT=xt_sb[:, c, ti * 128 : (ti + 1) * 128],
                    rhs=wv_sb[:, c, :],
                    start=(c == 0),
                    stop=(c == 5),
                )
            cpeng = nc.scalar if ti % 2 == 0 else nc.vector
            _copy(cpeng, v_sb[:, ti, :, :], vp.rearrange("p (h d) -> p h d", h=HPC))

    # h0 q/k projection as aux chunks (scalar copies: ACT idle here)
    for g in range(4):
        _proj_chunk(nc, aux, q_sb, wq_sb, xt_sb, 0, g,
                    nc.scalar if g % 2 == 0 else nc.vector, f"q0g{g}")
        _proj_chunk(nc, aux, k_sb, wk_sb, xt_sb, 0, g,
                    nc.scalar if g % 2 == 1 else nc.vector, f"k0g{g}")

    # deferred aux work queues
    aux_q = []
    for h in (1, 2):
        for g in range(4):
            aux_q.append(("proj", q_sb, wq_sb, h, g, f"q{h}g{g}"))
            aux_q.append(("proj", k_sb, wk_sb, h, g, f"k{h}g{g}"))

    def emit_aux(n):
        for _ in range(n):
            if not aux_q:
                return
            item = aux_q.pop(0)
            if item[0] == "proj":
                _, dst, wsb, h, g, name = item
                _proj_chunk(nc, aux, dst, wsb, xt_sb, h, g, nc.vector, name)
            else:
                _, ti, o, w, name = item
                _outproj_chunk(nc, aux, t_sb, wo_sb, osb, out, ti, o, w,
                               nc.vector, nc.gpsimd if ti % 2 else nc.sync, name)

    # ---------- attention ----------
    stp = ctx.enter_context(tc.tile_pool(name="stp", bufs=1, space="PSUM"))
    upp = ctx.enter_context(tc.tile_pool(name="upp", bufs=1, space="PSUM"))
    dnp = ctx.enter_context(tc.tile_pool(name="dnp", bufs=1, space="PSUM"))
    ptp = ctx.enter_context(tc.tile_pool(name="ptp", bufs=2))
    rsc = ctx.enter_context(tc.tile_pool(name="rsc", bufs=2))

    for half in range(2):
        for h in range(HPC):
            q0 = half * QH
            u_ps = [
                upp.tile([128, 512], fp32, tag=f"u{qc}", name=f"u{qc}") for qc in range(2)
            ]
            den = dnp.tile([128, 512], fp32, tag="den", name="den")
            pt_prev = None
            for ti in range(NT + 1):
                pt_cur = None
                if ti < NT:
                    st = stp.tile([128, 2 * QH], fp32, tag="st", name="st")
                    for qc in range(2):
                        qs = q0 + qc * 512
                        nc.tensor.matmul(
                            st[:, qc * 512 : (qc + 1) * 512],
                            lhsT=k_sb[0:64, h, ti * 128 : (ti + 1) * 128],
                            rhs=q_sb[0:64, h, qs : qs + 512],
                            start=True, stop=True, tile_position=(0, 0),
                        )
                        nc.tensor.matmul(
                            st[:, QH + qc * 512 : QH + (qc + 1) * 512],
                            lhsT=k_sb[64:128, h, ti * 128 : (ti + 1) * 128],
                            rhs=q_sb[64:128, h, qs : qs + 512],
                            start=True, stop=True, tile_position=(64, 0),
                        )
                    pt = ptp.tile([128, 2 * QH], bf16, tag="pt", name="pt")
                    if ti % DVE_MOD in DVE_SEL:
                        nc.scalar.activation(pt[:, 0:QH], st[:, 0:QH], Exp)
                        nc.vector.tensor_scalar(
                            pt[:, QH : 2 * QH].bitcast(F.int16),
                            st[:, QH : 2 * QH],
                            SCH_A, SCH_B, ALU.mult, ALU.add,
                        )
                    else:
                        nc.scalar.activation(pt, st, Exp)
                    pt_cur = pt
                if ti > 0:
                    tprev = ti - 1
                    stt = tprev == 0
                    stp_ = tprev == NT - 1
                    for qc in range(2):
                        nc.tensor.matmul(
                            u_ps[qc][0:64, :],
                            lhsT=v_sb[:, tprev, h, :],
                            rhs=pt_prev[:, qc * 512 : (qc + 1) * 512],
                            start=stt, stop=stp_, tile_position=(0, 0),
                            skip_group_check=True,
                        )
                        nc.tensor.matmul(
                            u_ps[qc][64:128, :],
                            lhsT=v_sb[:, tprev, h, :],
                            rhs=pt_prev[:, QH + qc * 512 : QH + (qc + 1) * 512],
                            start=stt, stop=stp_, tile_position=(0, 64),
                            skip_group_check=True,
                        )
                    for sl, br, qc in ((0, 0, 0), (32, 0, 1), (64, 1, 0), (96, 1, 1)):
                        nc.tensor.matmul(
                            den[sl : sl + 1, :],
                            lhsT=ones_sb,
                            rhs=pt_prev[:, br * QH + qc * 512 : br * QH + (qc + 1) * 512],
                            start=stt, stop=stp_, tile_position=(0, sl),
                            skip_group_check=True,
                        )
                    if tprev % 2 == 1:
                        emit_aux(1)
                pt_prev = pt_cur

            # ---- normalize: r = 1/den (+ lambda fold), broadcast, t = u*r ----
            dsc = rsc.tile([128, 512], fp32, tag="dsc", name="dsc")
            nc.vector.tensor_copy(dsc, den)
            d128 = rsc.tile([128, 16], fp32, tag="d128", name="d128")
            for r in range(4):
                nc.sync.dma_start(out=d128[r * 32 : (r + 1) * 32, :],
                                  in_=dsc[r * 32 : r * 32 + 1, :])
            r128 = rsc.tile([128, 16], fp32, tag="r128", name="r128")
            nc.vector.reciprocal(r128, d128)
            r128b = rsc.tile([128, 16], bf16, tag="r128b", name="r128b")
            nc.vector.tensor_scalar(r128b, r128, lam_sb[:, h : h + 1], None, ALU.mult)
            rrow = (h * 2 + half) * 4
            for r in range(4):
                nc.sync.dma_start(out=r_dram[rrow + r : rrow + r + 1, :],
                                  in_=r128b[r * 32 : (r + 1) * 32, :])
            rb = rsc.tile([128, QH], bf16, tag="rb", name="rb")
            for r, (pb, qc) in enumerate(((0, 0), (0, 1), (64, 0), (64, 1))):
                nc.sync.dma_start(
                    out=rb[pb : pb + 64, qc * 512 : (qc + 1) * 512],
                    in_=r_dram[rrow + r : rrow + r + 1, :].partition_broadcast(64),
                )
            for qc in range(2):
                nc.vector.tensor_tensor(
                    t_sb[:, h, q0 + qc * 512 : q0 + (qc + 1) * 512],
                    u_ps[qc], rb[:, qc * 512 : (qc + 1) * 512], ALU.mult,
                )

        # after each half completes: queue its output projection
        for ti in range(half * 8, half * 8 + 8):
            for o, w in ((0, 512), (512, 256)):
                aux_q.append(("outproj", ti, o, w, f"op{ti}o{o}"))
        if half == 1:
            # tail: flush remaining aux work with scalar copies (ACT idle)
            while aux_q:
                item = aux_q.pop(0)
                _, ti, o, w, name = item
                _outproj_chunk(nc, aux, t_sb, wo_sb, osb, out, ti, o, w,
                               nc.scalar if ti % 2 else nc.vector,
                               nc.gpsimd if ti % 2 else nc.sync, name)

    if taps:
        nc.sync.dma_start(out=taps["q"][:, :, :], in_=q_sb)
        nc.sync.dma_start(out=taps["k"][:, :, :], in_=k_sb)
        nc.sync.dma_start(out=taps["v"][:, :, :, :], in_=v_sb)
        nc.sync.dma_start(out=taps["t"][:, :, :], in_=t_sb)
        nc.sync.dma_start(out=taps["r"][:, :], in_=r_dram[:, :])


def build_bass(debug_taps=False):
    nc = bacc_mod.Bacc(None)
    xt = nc.dram_tensor("xt", [C, N], F.bfloat16, kind="ExternalInput")
    wq = nc.dram_tensor("wq", [C, HPC * 128], F.bfloat16, kind="ExternalInput")
    wk = nc.dram_tensor("wk", [C, HPC * 128], F.bfloat16, kind="ExternalInput")
    wv = nc.dram_tensor("wv", [C, HPC * D], F.bfloat16, kind="ExternalInput")
    wo = nc.dram_tensor("wo", [HPC * D, C], F.bfloat16, kind="ExternalInput")
    lamc = nc.dram_tensor("lamc", [128, HPC], F.float32, kind="ExternalInput")
    out = nc.dram_tensor("out", [N, C], F.bfloat16, kind="ExternalOutput")
    taps = None
    if debug_taps:
        taps = {
            "q": nc.dram_tensor("tap_q", [128, HPC, N], F.bfloat16, kind="ExternalOutput"),
            "k": nc.dram_tensor("tap_k", [128, HPC, N], F.bfloat16, kind="ExternalOutput"),
            "v": nc.dram_tensor("tap_v", [128, NT, HPC, D], F.bfloat16, kind="ExternalOutput"),
            "t": nc.dram_tensor("tap_t", [128, HPC, N], F.bfloat16, kind="ExternalOutput"),
            "r": nc.dram_tensor("tap_r", [HPC * 2 * 4, 512], F.bfloat16, kind="ExternalOutput"),
        }
    with TileContext(nc) as tc:
        with ExitStack() as ctx:
            _body(nc, tc, ctx, xt, wq, wk, wv, wo, lamc, out, taps=taps)
    nc.compile()
    return nc


_NC = None


def _get_nc():
    global _NC
    if _NC is None:
        _NC = build_bass()
    return _NC


def _prep_core(core, x, Wq, Wk, Wv, Wo, lam):
    b = core // 4
    heads = [(core % 4) * HPC + i for i in range(HPC)]
    sc = 1.0 / np.sqrt(D)
    xt = np.ascontiguousarray(x[b].T).astype(BF16)
    wq = np.empty((C, HPC * 128), np.float32)
    wk = np.empty((C, HPC * 128), np.float32)
    for i, h in enumerate(heads):
        for br in range(2):
            c0 = i * 128 + br * 64
            wq[:, c0 : c0 + 64] = Wq[:, br * C + h * D : br * C + (h + 1) * D] * sc
            wk[:, c0 : c0 + 64] = Wk[:, br * C + h * D : br * C + (h + 1) * D]
    wv = np.concatenate([Wv[:, h * D : (h + 1) * D] for h in heads], axis=1)
    wo = np.concatenate([Wo[h * D : (h + 1) * D, :] for h in heads], axis=0)
    lams = np.zeros((128, HPC), np.float32)
    for i, h in enumerate(heads):
        lams[0:64, i] = 1.0
        lams[64:128, i] = -lam[h]
    return dict(
        xt=xt,
        wq=wq.astype(BF16),
        wk=wk.astype(BF16),
        wv=wv.astype(BF16),
        wo=wo.astype(BF16),
        lamc=lams,
    )


def kernel(x, Wq, Wk, Wv, lambda_p, Wo, bo, _trace=False, _tmpdir=None, _taps=False):
    x = np.asarray(x, np.float32)
    lam = np.exp(np.asarray(lambda_p, np.float32).reshape(H))
    in_maps = [
        _prep_core(core, x, np.asarray(Wq, np.float32), np.asarray(Wk, np.float32),
                   np.asarray(Wv, np.float32), np.asarray(Wo, np.float32), lam)
        for core in range(NCORES)
    ]
    global _NC
    if _taps and _NC is None:
        _NC = build_bass(debug_taps=True)
    nc = _get_nc()
    res = run_bass_kernel_spmd(
        nc, in_maps, list(range(NCORES)), trace=_trace, tmpdir=_tmpdir
    )
    outf = np.zeros((B, N, C), np.float32)
    for core in range(NCORES):
        outf[core // 4] += np.asarray(res.results[core]["out"], np.float32)
    outf += np.asarray(bo, np.float32)[None, None, :]
    if _trace:
        kernel.last_exec_time_ns = res.exec_time_ns
    kernel.last_results = res.results
    return outf


# revision 6
# speedup vs baseline: 1.3254x; 1.3254x over previous
"""Differential attention kernel for Trainium2, 8 NeuronCores — v3.

Sharding: B(2) x head-groups(4) -> 8 cores; each core computes 3 heads'
differential attention for one batch element plus its partial slice of the
output projection (row-parallel over Wo). Host sums the 4 partials per batch
element and adds bo.

Design:
  * Scores: the two branches of a head run as a row-tiled PE pair
    (K=64 tiles at tile_position (0,0)/(64,0)) -> 2x score throughput.
    q_sb/k_sb per head hold branch0 on partitions 0:64, branch1 on 64:128.
  * PV: col-tiled pair (M=64 tiles at (0,0)/(0,64)) -> u bank rows 0:64 =
    branch0, rows 64:128 = branch1. Softmax denominators come from four M=1
    ones-matmuls packed at col positions 0/32/64/96 of one bank, accumulated
    across key strips (cost: one extra N-stream per strip for all four).
  * exp split: branch0 -> ScalarE exact exp; branch1 on every other strip ->
    VectorE Schraudolph (tensor_scalar fp32 -> int16 bits of bf16 exp).
  * u is cast to SBUF right after the last PV so the PSUM banks free
    immediately; normalization (t = u * (1/den broadcast)) runs on SBUF and
    never blocks the next head's attention.
  * -lambda is folded into the Wo rows of branch1 on the host; the output
    projection contracts K=128 over duplicated/scaled Wo rows, folding the
    branch combine into the matmul.
  * qk projections of heads 1,2 and the half-0 output projection are emitted
    interleaved into the attention stream (one aux PSUM bank) so the PE has
    fill work while ACT/DVE run exp; the half-1 output projection runs as a
    tail phase with a 4-buffer PSUM pool after the attention pools close.
"""

import os
import sys
from contextlib import ExitStack

for _p in ("/opt/trn_rl_repo", "/root/.axon_site/_ro/trn_rl_repo"):
    if os.path.isdir(_p) and _p not in sys.path:
        sys.path.insert(0, _p)

import ml_dtypes
import numpy as np

import concourse.bass as bass
import concourse.bacc as bacc_mod
import concourse.mybir as mybir
from concourse.bass_utils import run_bass_kernel_spmd
from concourse.tile import TileContext

BF16 = ml_dtypes.bfloat16
F = mybir.dt
ALU = mybir.AluOpType

B, N, C, H, D = 2, 2048, 768, 12, 64
HPC = 3          # heads per core
NCORES = 8
NT = N // 128    # 16 key strips
QH = 1024        # q processed per (head, half)

# Schraudolph exp: bf16 bits = rint(s * 128*log2(e) + (16256 - corr))
SCH_A = 128.0 * 1.4426950408889634
SCH_B = 16256.0 - 4.0
# strips whose branch-1 exp goes to the DVE (ti % DVE_MOD in DVE_SEL)
DVE_MOD = 2
DVE_SEL = (1,)


def _copy(eng, out, in_):
    if hasattr(eng, "tensor_copy"):
        eng.tensor_copy(out, in_)
    else:
        eng.copy(out, in_)


def _proj_chunk(nc, pool, dst, w_sb, xt_sb, h, g, cpeng, name):
    """One 512-wide qk-projection chunk: dst[:, h, g*512:+512]."""
    pp = pool.tile([128, 512], F.float32, tag="aux", name=f"aux_{name}")
    for c in range(6):
        nc.tensor.matmul(
            pp,
            lhsT=w_sb[:, c, h * 128 : (h + 1) * 128],
            rhs=xt_sb[:, c, g * 512 : (g + 1) * 512],
            start=(c == 0),
            stop=(c == 5),
        )
    _copy(cpeng, dst[:, h, g * 512 : (g + 1) * 512], pp)


def _outproj_chunk(nc, pool, tag, t_sb, wo_sb, osb, out, ti, o, w, cpeng, dmaeng, name):
    """Output projection for row strip ti, out cols [o, o+w): K=128 over
    duplicated-wo rows folds the two-branch combine."""
    fo = pool.tile([128, 512], F.float32, tag=tag, name=f"fo_{name}")
    for hh in range(HPC):
        nc.tensor.matmul(
            fo[:, 0:w],
            lhsT=t_sb[:, hh, ti * 128 : (ti + 1) * 128],
            rhs=wo_sb[:, hh, o : o + w],
            start=(hh == 0),
            stop=(hh == HPC - 1),
        )
    ot = osb.tile([128, 512], F.bfloat16, tag="osb", name=f"osb_{name}")
    _copy(cpeng, ot[:, 0:w], fo[:, 0:w])
    dmaeng.dma_start(out=out[ti * 128 : (ti + 1) * 128, o : o + w], in_=ot[:, 0:w])


def _body(nc, tc, ctx, xt, wq, wk, wv, wo, out, taps=None):
    fp32, bf16 = F.float32, F.bfloat16
    Exp = mybir.ActivationFunctionType.Exp

    singles = ctx.enter_context(tc.tile_pool(name="singles", bufs=1))
    wo_sb = singles.tile([128, HPC, C], bf16)
    ones_sb = singles.tile([128, 1], bf16)
    xt_sb = singles.tile([128, 6, N], bf16)
    wq_sb = singles.tile([128, 6, HPC * 128], bf16)
    wk_sb = singles.tile([128, 6, HPC * 128], bf16)
    wv_sb = singles.tile([128, 6, HPC * D], bf16)
    q_sb = singles.tile([128, HPC, N], bf16)
    k_sb = singles.tile([128, HPC, N], bf16)
    v_sb = singles.tile([128, NT, HPC, D], bf16)
    t_sb = singles.tile([128, HPC, N], bf16)
    # r bounce rows: [(h*2+half)*2 + br] -> [qc0 512 | qc1 512]
    r_dram = nc.dram_tensor("r_bounce", [HPC * 2 * 2, QH], F.bfloat16)

    nc.vector.memset(ones_sb, 1.0)
    # wo rows duplicated to both partition halves (branch1 rows carry -lambda,
    # folded on the host into wo[64:128, ...])
    wo_r = wo[:, :].rearrange("(l h d) c -> l h d c", l=2, h=HPC)
    for h in range(HPC):
        nc.sync.dma_start(out=wo_sb[0:64, h, :], in_=wo_r[0, h, :, :])
        nc.gpsimd.dma_start(out=wo_sb[64:128, h, :], in_=wo_r[1, h, :, :])
    xt_r = xt[:, :].rearrange("(ch p) n -> p ch n", p=128)
    wq_r = wq[:, :].rearrange("(ch p) w -> p ch w", p=128)
    wk_r = wk[:, :].rearrange("(ch p) w -> p ch w", p=128)
    wv_r = wv[:, :].rearrange("(ch p) w -> p ch w", p=128)
    for c in range(6):
        nc.sync.dma_start(out=wv_sb[:, c, :], in_=wv_r[:, c, :])
    for c in range(6):
        eng = nc.sync if c % 2 == 0 else nc.gpsimd
        eng.dma_start(out=xt_sb[:, c, :], in_=xt_r[:, c, :])
    for c in range(6):
        nc.sync.dma_start(out=wq_sb[:, c, :], in_=wq_r[:, c, :])
        nc.gpsimd.dma_start(out=wk_sb[:, c, :], in_=wk_r[:, c, :])

    # pre-warm the PE clock gate during the initial DMA wait
    with tc.tile_pool(name="warm_sb", bufs=1) as warm_sb, \
         tc.tile_pool(name="warm_ps", bufs=1, space="PSUM") as warm_ps:
        wsrc = warm_sb.tile([128, 512], bf16)
        nc.vector.memset(wsrc, 0.0)
        wt = warm_ps.tile([128, 512], fp32)
        for _ in range(48):
            nc.tensor.matmul(wt, lhsT=wsrc[:, 0:128], rhs=wsrc, start=True, stop=True)

    osb = ctx.enter_context(tc.tile_pool(name="osb", bufs=3))

    with tc.tile_pool(name="aux", bufs=1, space="PSUM") as aux:
        # ---------- v projection + head-0 qk projection (phase) ----------
        with tc.tile_pool(name="vpp", bufs=2, space="PSUM") as vpp:
            for ti in range(NT):
                vp = vpp.tile([128, HPC * D], fp32)
                for c in range(6):
                    nc.tensor.matmul(
                        vp,
                        lhsT=xt_sb[:, c, ti * 128 : (ti + 1) * 128],
                        rhs=wv_sb[:, c, :],
                        start=(c == 0),
                        stop=(c == 5),
                    )
                cpeng = nc.scalar if ti % 2 == 0 else nc.vector
                _copy(cpeng, v_sb[:, ti, :, :], vp.rearrange("p (h d) -> p h d", h=HPC))

        # h0 q/k projection as aux chunks (scalar copies: ACT idle here)
        for g in range(4):
            _proj_chunk(nc, aux, q_sb, wq_sb, xt_sb, 0, g,
                        nc.scalar if g % 2 == 0 else nc.vector, f"q0g{g}")
            _proj_chunk(nc, aux, k_sb, wk_sb, xt_sb, 0, g,
                        nc.scalar if g % 2 == 1 else nc.vector, f"k0g{g}")

        # deferred aux work queue
        aux_q = []
        for h in (1, 2):
            for g in range(4):
                aux_q.append(("proj", q_sb, wq_sb, h, g, f"q{h}g{g}"))
                aux_q.append(("proj", k_sb, wk_sb, h, g, f"k{h}g{g}"))

        def emit_aux(n):
            for _ in range(n):
                if not aux_q:
                    return
                item = aux_q.pop(0)
                if item[0] == "proj":
                    _, dst, wsb, h, g, name = item
                    _proj_chunk(nc, aux, dst, wsb, xt_sb, h, g, nc.vector, name)
                else:
                    _, ti, o, w, name = item
                    _outproj_chunk(nc, aux, "aux", t_sb, wo_sb, osb, out, ti, o, w,
                                   nc.vector, nc.gpsimd if ti % 2 else nc.sync, name)

        # ---------- attention ----------
        with tc.tile_pool(name="stp", bufs=1, space="PSUM") as stp, \
             tc.tile_pool(name="upp", bufs=1, space="PSUM") as upp, \
             tc.tile_pool(name="dnp", bufs=1, space="PSUM") as dnp, \
             tc.tile_pool(name="ptp", bufs=2) as ptp, \
             tc.tile_pool(name="rsc", bufs=2) as rsc:
            for half in range(2):
                for h in range(HPC):
                    q0 = half * QH
                    u_ps = upp.tile([128, 2 * 512], fp32, tag="u", name="u")
                    den = dnp.tile([128, 512], fp32, tag="den", name="den")
                    pt_prev = None
                    for ti in range(NT + 1):
                        pt_cur = None
                        if ti < NT:
                            st = stp.tile([128, 2 * QH], fp32, tag="st", name="st")
                            for qc in range(2):
                                qs = q0 + qc * 512
                                nc.tensor.matmul(
                                    st[:, qc * 512 : (qc + 1) * 512],
                                    lhsT=k_sb[0:64, h, ti * 128 : (ti + 1) * 128],
                                    rhs=q_sb[0:64, h, qs : qs + 512],
                                    start=True, stop=True, tile_position=(0, 0),
                                )
                                nc.tensor.matmul(
                                    st[:, QH + qc * 512 : QH + (qc + 1) * 512],
                                    lhsT=k_sb[64:128, h, ti * 128 : (ti + 1) * 128],
                                    rhs=q_sb[64:128, h, qs : qs + 512],
                                    start=True, stop=True, tile_position=(64, 0),
                                )
                            pt = ptp.tile([128, 2 * QH], bf16, tag="pt", name="pt")
                            if ti % DVE_MOD in DVE_SEL:
                                nc.scalar.activation(pt[:, 0:QH], st[:, 0:QH], Exp)
                                nc.vector.tensor_scalar(
                                    pt[:, QH : 2 * QH].bitcast(F.int16),
                                    st[:, QH : 2 * QH],
                                    SCH_A, SCH_B, ALU.mult, ALU.add,
                                )
                            else:
                                nc.scalar.activation(pt, st, Exp)
                            pt_cur = pt
                        if ti > 0:
                            tprev = ti - 1
                            stt = tprev == 0
                            stp_ = tprev == NT - 1
                            for qc in range(2):
                                nc.tensor.matmul(
                                    u_ps[0:64, qc * 512 : (qc + 1) * 512],
                                    lhsT=v_sb[:, tprev, h, :],
                                    rhs=pt_prev[:, qc * 512 : (qc + 1) * 512],
                                    start=stt, stop=stp_, tile_position=(0, 0),
                                    skip_group_check=True,
                                )
                                nc.tensor.matmul(
                                    u_ps[64:128, qc * 512 : (qc + 1) * 512],
                                    lhsT=v_sb[:, tprev, h, :],
                                    rhs=pt_prev[:, QH + qc * 512 : QH + (qc + 1) * 512],
                                    start=stt, stop=stp_, tile_position=(0, 64),
                                    skip_group_check=True,
                                )
                            for sl, br, qc in ((0, 0, 0), (32, 0, 1), (64, 1, 0), (96, 1, 1)):
                                nc.tensor.matmul(
                                    den[sl : sl + 1, :],
                                    lhsT=ones_sb,
                                    rhs=pt_prev[:, br * QH + qc * 512 : br * QH + (qc + 1) * 512],
                                    start=stt, stop=stp_, tile_position=(0, sl),
                                    skip_group_check=True,
                                )
                            if tprev % 2 == 1:
                                emit_aux(1)
                        pt_prev = pt_cur

                    # ---- free u: cast to SBUF right away ----
                    u_sb = rsc.tile([128, QH], bf16, tag="usb", name="u_sb")
                    nc.vector.tensor_copy(u_sb, u_ps)
                    # ---- r = 1/den, broadcast via DRAM bounce ----
                    dsc = rsc.tile([128, 512], fp32, tag="dsc", name="dsc")
                    nc.vector.tensor_copy(dsc, den)
                    d128 = rsc.tile([128, 16], fp32, tag="d128", name="d128")
                    for r in range(4):
                        eng = nc.sync if r % 2 == 0 else nc.gpsimd
                        eng.dma_start(out=d128[r * 32 : (r + 1) * 32, :],
                                      in_=dsc[r * 32 : r * 32 + 1, :])
                    r128 = rsc.tile([128, 16], fp32, tag="r128", name="r128")
                    nc.vector.reciprocal(r128, d128)
                    r128b = rsc.tile([128, 16], bf16, tag="r128b", name="r128b")
                    nc.vector.tensor_copy(r128b, r128)
                    rrow = (h * 2 + half) * 2
                    nc.sync.dma_start(out=r_dram[rrow : rrow + 1, :], in_=r128b[0:64, :])
                    nc.gpsimd.dma_start(out=r_dram[rrow + 1 : rrow + 2, :], in_=r128b[64:128, :])
                    rb = rsc.tile([128, QH], bf16, tag="rb", name="rb")
                    nc.sync.dma_start(
                        out=rb[0:64, :],
                        in_=r_dram[rrow : rrow + 1, :].partition_broadcast(64),
                    )
                    nc.gpsimd.dma_start(
                        out=rb[64:128, :],
                        in_=r_dram[rrow + 1 : rrow + 2, :].partition_broadcast(64),
                    )
                    # qc slices of u_sb are [b0 qc | b1 qc] stacked on partitions;
                    # rb rows 0:64 = r_b0, 64:128 = r_b1 per qc
                    nc.vector.tensor_tensor(
                        t_sb[:, h, q0 : q0 + QH], u_sb, rb, ALU.mult,
                    )

                # after each half: queue/run its output projection
                if half == 0:
                    for ti in range(8):
                        for o, w in ((0, 512), (512, 256)):
                            aux_q.append(("outproj", ti, o, w, f"op{ti}o{o}"))
        # flush any aux leftovers before the pool closes
        emit_aux(len(aux_q))

    # ---------- tail: half-1 output projection with a wide pool ----------
    with tc.tile_pool(name="fop", bufs=4, space="PSUM") as fop:
        for i, ti in enumerate(range(8, 16)):
            for o, w in ((0, 512), (512, 256)):
                cpeng = nc.scalar if (i + (o > 0)) % 2 == 0 else nc.vector
                dmaeng = nc.gpsimd if ti % 2 else nc.sync
                _outproj_chunk(nc, fop, "fo", t_sb, wo_sb, osb, out, ti, o, w,
                               cpeng, dmaeng, f"tp{ti}o{o}")

    if taps:
        nc.sync.dma_start(out=taps["q"][:, :, :], in_=q_sb)
        nc.sync.dma_start(out=taps["k"][:, :, :], in_=k_sb)
        nc.sync.dma_start(out=taps["v"][:, :, :, :], in_=v_sb)
        nc.sync.dma_start(out=taps["t"][:, :, :], in_=t_sb)
        nc.sync.dma_start(out=taps["r"][:, :], in_=r_dram[:, :])


def build_bass(debug_taps=False):
    nc = bacc_mod.Bacc(None)
    xt = nc.dram_tensor("xt", [C, N], F.bfloat16, kind="ExternalInput")
    wq = nc.dram_tensor("wq", [C, HPC * 128], F.bfloat16, kind="ExternalInput")
    wk = nc.dram_tensor("wk", [C, HPC * 128], F.bfloat16, kind="ExternalInput")
    wv = nc.dram_tensor("wv", [C, HPC * D], F.bfloat16, kind="ExternalInput")
    wo = nc.dram_tensor("wo", [2 * HPC * D, C], F.bfloat16, kind="ExternalInput")
    out = nc.dram_tensor("out", [N, C], F.bfloat16, kind="ExternalOutput")
    taps = None
    if debug_taps:
        taps = {
            "q": nc.dram_tensor("tap_q", [128, HPC, N], F.bfloat16, kind="ExternalOutput"),
            "k": nc.dram_tensor("tap_k", [128, HPC, N], F.bfloat16, kind="ExternalOutput"),
            "v": nc.dram_tensor("tap_v", [128, NT, HPC, D], F.bfloat16, kind="ExternalOutput"),
            "t": nc.dram_tensor("tap_t", [128, HPC, N], F.bfloat16, kind="ExternalOutput"),
            "r": nc.dram_tensor("tap_r", [HPC * 2 * 2, QH], F.bfloat16, kind="ExternalOutput"),
        }
    with TileContext(nc) as tc:
        with ExitStack() as ctx:
            _body(nc, tc, ctx, xt, wq, wk, wv, wo, out, taps=taps)
    nc.compile()
    return nc


_NC = None


def _get_nc():
    global _NC
    if _NC is None:
        _NC = build_bass()
    return _NC


def _prep_core(core, x, Wq, Wk, Wv, Wo, lam):
    b = core // 4
    heads = [(core % 4) * HPC + i for i in range(HPC)]
    sc = 1.0 / np.sqrt(D)
    xt = np.ascontiguousarray(x[b].T).astype(BF16)
    wq = np.empty((C, HPC * 128), np.float32)
    wk = np.empty((C, HPC * 128), np.float32)
    for i, h in enumerate(heads):
        for br in range(2):
            c0 = i * 128 + br * 64
            wq[:, c0 : c0 + 64] = Wq[:, br * C + h * D : br * C + (h + 1) * D] * sc
            wk[:, c0 : c0 + 64] = Wk[:, br * C + h * D : br * C + (h + 1) * D]
    wv = np.concatenate([Wv[:, h * D : (h + 1) * D] for h in heads], axis=1)
    # wo: first 192 rows = plain Wo rows (branch0 / partitions 0:64 dup),
    # next 192 rows = -lambda_h * Wo rows (branch1 / partitions 64:128 dup)
    wo0 = np.concatenate([Wo[h * D : (h + 1) * D, :] for h in heads], axis=0)
    wo1 = np.concatenate(
        [-lam[h] * Wo[h * D : (h + 1) * D, :] for h in heads], axis=0
    )
    wo = np.concatenate([wo0, wo1], axis=0)
    return dict(
        xt=xt,
        wq=wq.astype(BF16),
        wk=wk.astype(BF16),
        wv=wv.astype(BF16),
        wo=wo.astype(BF16),
    )


def kernel(x, Wq, Wk, Wv, lambda_p, Wo, bo, _trace=False, _tmpdir=None, _taps=False):
    x = np.asarray(x, np.float32)
    lam = np.exp(np.asarray(lambda_p, np.float32).reshape(H))
    in_maps = [
        _prep_core(core, x, np.asarray(Wq, np.float32), np.asarray(Wk, np.float32),
                   np.asarray(Wv, np.float32), np.asarray(Wo, np.float32), lam)
        for core in range(NCORES)
    ]
    global _NC
    if _taps and _NC is None:
        _NC = build_bass(debug_taps=True)
    nc = _get_nc()
    res = run_bass_kernel_spmd(
        nc, in_maps, list(range(NCORES)), trace=_trace, tmpdir=_tmpdir
    )
    outf = np.zeros((B, N, C), np.float32)
    for core in range(NCORES):
        outf[core // 4] += np.asarray(res.results[core]["out"], np.float32)
    outf += np.asarray(bo, np.float32)[None, None, :]
    if _trace:
        kernel.last_exec_time_ns = res.exec_time_ns
    kernel.last_results = res.results
    return outf


# revision 7
# speedup vs baseline: 1.5559x; 1.1739x over previous
"""Differential attention kernel for Trainium2, 8 NeuronCores — v3.

Sharding: B(2) x head-groups(4) -> 8 cores; each core computes 3 heads'
differential attention for one batch element plus its partial slice of the
output projection (row-parallel over Wo). Host sums the 4 partials per batch
element and adds bo.

Design:
  * Scores: the two branches of a head run as a row-tiled PE pair
    (K=64 tiles at tile_position (0,0)/(64,0)) -> 2x score throughput.
    q_sb/k_sb per head hold branch0 on partitions 0:64, branch1 on 64:128.
  * PV: col-tiled pair (M=64 tiles at (0,0)/(0,64)) -> u bank rows 0:64 =
    branch0, rows 64:128 = branch1. Softmax denominators come from four M=1
    ones-matmuls packed at col positions 0/32/64/96 of one bank, accumulated
    across key strips (cost: one extra N-stream per strip for all four).
  * exp split: branch0 -> ScalarE exact exp; branch1 on every other strip ->
    VectorE Schraudolph (tensor_scalar fp32 -> int16 bits of bf16 exp).
  * u is cast to SBUF right after the last PV so the PSUM banks free
    immediately; normalization (t = u * (1/den broadcast)) runs on SBUF and
    never blocks the next head's attention.
  * -lambda is folded into the Wo rows of branch1 on the host; the output
    projection contracts K=128 over duplicated/scaled Wo rows, folding the
    branch combine into the matmul.
  * qk projections of heads 1,2 and the half-0 output projection are emitted
    interleaved into the attention stream (one aux PSUM bank) so the PE has
    fill work while ACT/DVE run exp; the half-1 output projection runs as a
    tail phase with a 4-buffer PSUM pool after the attention pools close.
"""

import os
import sys
from contextlib import ExitStack

for _p in ("/opt/trn_rl_repo", "/root/.axon_site/_ro/trn_rl_repo"):
    if os.path.isdir(_p) and _p not in sys.path:
        sys.path.insert(0, _p)

import ml_dtypes
import numpy as np

import concourse.bass as bass
import concourse.bacc as bacc_mod
import concourse.mybir as mybir
from concourse.bass_utils import run_bass_kernel_spmd
from concourse.tile import TileContext

BF16 = ml_dtypes.bfloat16
F = mybir.dt
ALU = mybir.AluOpType

B, N, C, H, D = 2, 2048, 768, 12, 64
HPC = 3          # heads per core
NCORES = 8
NT = N // 128    # 16 key strips
QH = 1024        # q processed per (head, half)

# Schraudolph exp: bf16 bits = rint(s * 128*log2(e) + (16256 - corr))
SCH_A = 128.0 * 1.4426950408889634
SCH_B = 16256.0 - 4.0
# strips whose branch-1 exp goes to the DVE (ti % DVE_MOD in DVE_SEL)
DVE_MOD = 2
DVE_SEL = (1,)


def _copy(eng, out, in_):
    if hasattr(eng, "tensor_copy"):
        eng.tensor_copy(out, in_)
    else:
        eng.copy(out, in_)


def _proj_chunk(nc, pool, dst, w_sb, xt_sb, h, g, cpeng, name):
    """One 512-wide qk-projection chunk: dst[:, h, g*512:+512]."""
    pp = pool.tile([128, 512], F.float32, tag="aux", name=f"aux_{name}")
    for c in range(6):
        nc.tensor.matmul(
            pp,
            lhsT=w_sb[:, c, h * 128 : (h + 1) * 128],
            rhs=xt_sb[:, c, g * 512 : (g + 1) * 512],
            start=(c == 0),
            stop=(c == 5),
        )
    _copy(cpeng, dst[:, h, g * 512 : (g + 1) * 512], pp)


def _outproj_chunk(nc, pool, tag, t_sb, wo_sb, osb, out, ti, o, w, cpeng, dmaeng, name):
    """Output projection for row strip ti, out cols [o, o+w): K=128 over
    duplicated-wo rows folds the two-branch combine."""
    fo = pool.tile([128, 512], F.float32, tag=tag, name=f"fo_{name}")
    for hh in range(HPC):
        nc.tensor.matmul(
            fo[:, 0:w],
            lhsT=t_sb[:, hh, ti * 128 : (ti + 1) * 128],
            rhs=wo_sb[:, hh, o : o + w],
            start=(hh == 0),
            stop=(hh == HPC - 1),
        )
    ot = osb.tile([128, 512], F.bfloat16, tag="osb", name=f"osb_{name}")
    _copy(cpeng, ot[:, 0:w], fo[:, 0:w])
    dmaeng.dma_start(out=out[ti * 128 : (ti + 1) * 128, o : o + w], in_=ot[:, 0:w])


def _body(nc, tc, ctx, xt, wq, wk, wv, wo, out, taps=None):
    fp32, bf16 = F.float32, F.bfloat16
    Exp = mybir.ActivationFunctionType.Exp

    singles = ctx.enter_context(tc.tile_pool(name="singles", bufs=1))
    wo_sb = singles.tile([128, HPC, C], bf16)
    ones_sb = singles.tile([128, 1], bf16)
    xt_sb = singles.tile([128, 6, N], bf16)
    wq_sb = singles.tile([128, 6, HPC * 128], bf16)
    wk_sb = singles.tile([128, 6, HPC * 128], bf16)
    wv_sb = singles.tile([128, 6, HPC * D], bf16)
    q_sb = singles.tile([128, HPC, N], bf16)
    k_sb = singles.tile([128, HPC, N], bf16)
    v_sb = singles.tile([128, NT, HPC, D], bf16)
    t_sb = singles.tile([128, HPC, N], bf16)
    # r bounce rows: [(h*2+half)*2 + br] -> [qc0 512 | qc1 512]
    r_dram = nc.dram_tensor("r_bounce", [HPC * 2 * 2, QH], F.bfloat16)

    nc.vector.memset(ones_sb, 1.0)
    # wo rows duplicated to both partition halves (branch1 rows carry -lambda,
    # folded on the host into wo[64:128, ...])
    wo_r = wo[:, :].rearrange("(l h d) c -> l h d c", l=2, h=HPC)
    for h in range(HPC):
        nc.sync.dma_start(out=wo_sb[0:64, h, :], in_=wo_r[0, h, :, :])
        nc.gpsimd.dma_start(out=wo_sb[64:128, h, :], in_=wo_r[1, h, :, :])
    xt_r = xt[:, :].rearrange("(ch p) n -> p ch n", p=128)
    wq_r = wq[:, :].rearrange("(ch p) w -> p ch w", p=128)
    wk_r = wk[:, :].rearrange("(ch p) w -> p ch w", p=128)
    wv_r = wv[:, :].rearrange("(ch p) w -> p ch w", p=128)
    for c in range(6):
        nc.sync.dma_start(out=wv_sb[:, c, :], in_=wv_r[:, c, :])
    for c in range(6):
        eng = nc.sync if c % 2 == 0 else nc.gpsimd
        eng.dma_start(out=xt_sb[:, c, :], in_=xt_r[:, c, :])
    for c in range(6):
        nc.sync.dma_start(out=wq_sb[:, c, :], in_=wq_r[:, c, :])
        nc.gpsimd.dma_start(out=wk_sb[:, c, :], in_=wk_r[:, c, :])

    # pre-warm the PE clock gate during the initial DMA wait
    with tc.tile_pool(name="warm_sb", bufs=1) as warm_sb, \
         tc.tile_pool(name="warm_ps", bufs=1, space="PSUM") as warm_ps:
        wsrc = warm_sb.tile([128, 512], bf16)
        nc.vector.memset(wsrc, 0.0)
        wt = warm_ps.tile([128, 512], fp32)
        for _ in range(64):
            nc.tensor.matmul(wt, lhsT=wsrc[:, 0:128], rhs=wsrc, start=True, stop=True)

    osb = ctx.enter_context(tc.tile_pool(name="osb", bufs=3))

    with tc.tile_pool(name="aux", bufs=1, space="PSUM") as aux:
        # ---------- v projection + head-0 qk projection (phase) ----------
        with tc.tile_pool(name="vpp", bufs=2, space="PSUM") as vpp:
            for ti in range(NT):
                vp = vpp.tile([128, HPC * D], fp32)
                for c in range(6):
                    nc.tensor.matmul(
                        vp,
                        lhsT=xt_sb[:, c, ti * 128 : (ti + 1) * 128],
                        rhs=wv_sb[:, c, :],
                        start=(c == 0),
                        stop=(c == 5),
                    )
                cpeng = nc.scalar if ti % 2 == 0 else nc.vector
                _copy(cpeng, v_sb[:, ti, :, :], vp.rearrange("p (h d) -> p h d", h=HPC))

        # h0 q/k projection as aux chunks (scalar copies: ACT idle here)
        for g in range(4):
            _proj_chunk(nc, aux, q_sb, wq_sb, xt_sb, 0, g,
                        nc.scalar if g % 2 == 0 else nc.vector, f"q0g{g}")
            _proj_chunk(nc, aux, k_sb, wk_sb, xt_sb, 0, g,
                        nc.scalar if g % 2 == 1 else nc.vector, f"k0g{g}")

        # deferred aux work queue
        aux_q = []
        for h in (1, 2):
            for g in range(4):
                aux_q.append(("proj", q_sb, wq_sb, h, g, f"q{h}g{g}"))
                aux_q.append(("proj", k_sb, wk_sb, h, g, f"k{h}g{g}"))

        def emit_aux(n):
            for _ in range(n):
                if not aux_q:
                    return
                item = aux_q.pop(0)
                if item[0] == "proj":
                    _, dst, wsb, h, g, name = item
                    _proj_chunk(nc, aux, dst, wsb, xt_sb, h, g, nc.vector, name)
                else:
                    _, ti, o, w, name = item
                    _outproj_chunk(nc, aux, "aux", t_sb, wo_sb, osb, out, ti, o, w,
                                   nc.vector, nc.gpsimd if ti % 2 else nc.sync, name)

        # ---------- attention ----------
        with tc.tile_pool(name="stp", bufs=1, space="PSUM") as stp, \
             tc.tile_pool(name="upp", bufs=1, space="PSUM") as upp, \
             tc.tile_pool(name="dnp", bufs=1, space="PSUM") as dnp, \
             tc.tile_pool(name="ptp", bufs=2) as ptp, \
             tc.tile_pool(name="rsc", bufs=2) as rsc:
            for half in range(2):
                for h in range(HPC):
                    q0 = half * QH
                    u_ps = upp.tile([128, 2 * 512], fp32, tag="u", name="u")
                    den = dnp.tile([128, 512], fp32, tag="den", name="den")
                    pt_prev = None
                    for ti in range(NT + 1):
                        pt_cur = None
                        if ti < NT:
                            st0 = stp.tile([128, QH], fp32, tag="st0", name="st0")
                            st1 = stp.tile([128, QH], fp32, tag="st1", name="st1")
                            for qc in range(2):
                                qs = q0 + qc * 512
                                nc.tensor.matmul(
                                    st0[:, qc * 512 : (qc + 1) * 512],
                                    lhsT=k_sb[0:64, h, ti * 128 : (ti + 1) * 128],
                                    rhs=q_sb[0:64, h, qs : qs + 512],
                                    start=True, stop=True, tile_position=(0, 0),
                                )
                                nc.tensor.matmul(
                                    st1[:, qc * 512 : (qc + 1) * 512],
                                    lhsT=k_sb[64:128, h, ti * 128 : (ti + 1) * 128],
                                    rhs=q_sb[64:128, h, qs : qs + 512],
                                    start=True, stop=True, tile_position=(64, 0),
                                )
                            pt0 = ptp.tile([128, QH], bf16, tag="pt0", name="pt0")
                            pt1 = ptp.tile([128, QH], bf16, tag="pt1", name="pt1")
                            nc.scalar.activation(pt0, st0, Exp)
                            if ti % DVE_MOD in DVE_SEL:
                                nc.vector.tensor_scalar(
                                    pt1[:, :].bitcast(F.int16), st1,
                                    SCH_A, SCH_B, ALU.mult, ALU.add,
                                )
                            else:
                                nc.scalar.activation(pt1, st1, Exp)
                            pt_cur = (pt0, pt1)
                        if ti > 0:
                            tprev = ti - 1
                            stt = tprev == 0
                            stp_ = tprev == NT - 1
                            for qc in range(2):
                                nc.tensor.matmul(
                                    u_ps[0:64, qc * 512 : (qc + 1) * 512],
                                    lhsT=v_sb[:, tprev, h, :],
                                    rhs=pt_prev[0][:, qc * 512 : (qc + 1) * 512],
                                    start=stt, stop=stp_, tile_position=(0, 0),
                                    skip_group_check=True,
                                )
                                nc.tensor.matmul(
                                    u_ps[64:128, qc * 512 : (qc + 1) * 512],
                                    lhsT=v_sb[:, tprev, h, :],
                                    rhs=pt_prev[1][:, qc * 512 : (qc + 1) * 512],
                                    start=stt, stop=stp_, tile_position=(0, 64),
                                    skip_group_check=True,
                                )
                            for sl, br, qc in ((0, 0, 0), (32, 0, 1), (64, 1, 0), (96, 1, 1)):
                                nc.tensor.matmul(
                                    den[sl : sl + 1, :],
                                    lhsT=ones_sb,
                                    rhs=pt_prev[br][:, qc * 512 : (qc + 1) * 512],
                                    start=stt, stop=stp_, tile_position=(0, sl),
                                    skip_group_check=True,
                                )
                            if tprev % 2 == 1:
                                emit_aux(1)
                        pt_prev = pt_cur

                    # ---- free u: cast to SBUF right away ----
                    u_sb = rsc.tile([128, QH], bf16, tag="usb", name="u_sb")
                    nc.vector.tensor_copy(u_sb, u_ps)
                    # ---- r = 1/den, broadcast via DRAM bounce ----
                    dsc = rsc.tile([128, 512], fp32, tag="dsc", name="dsc")
                    nc.vector.tensor_copy(dsc, den)
                    d128 = rsc.tile([128, 16], fp32, tag="d128", name="d128")
                    for r in range(4):
                        eng = nc.sync if r % 2 == 0 else nc.gpsimd
                        eng.dma_start(out=d128[r * 32 : (r + 1) * 32, :],
                                      in_=dsc[r * 32 : r * 32 + 1, :])
                    r128 = rsc.tile([128, 16], fp32, tag="r128", name="r128")
                    nc.vector.reciprocal(r128, d128)
                    r128b = rsc.tile([128, 16], bf16, tag="r128b", name="r128b")
                    nc.vector.tensor_copy(r128b, r128)
                    rrow = (h * 2 + half) * 2
                    nc.sync.dma_start(out=r_dram[rrow : rrow + 1, :], in_=r128b[0:64, :])
                    nc.gpsimd.dma_start(out=r_dram[rrow + 1 : rrow + 2, :], in_=r128b[64:128, :])
                    rb = rsc.tile([128, QH], bf16, tag="rb", name="rb")
                    nc.sync.dma_start(
                        out=rb[0:64, :],
                        in_=r_dram[rrow : rrow + 1, :].partition_broadcast(64),
                    )
                    nc.gpsimd.dma_start(
                        out=rb[64:128, :],
                        in_=r_dram[rrow + 1 : rrow + 2, :].partition_broadcast(64),
                    )
                    # qc slices of u_sb are [b0 qc | b1 qc] stacked on partitions;
                    # rb rows 0:64 = r_b0, 64:128 = r_b1 per qc
                    nc.vector.tensor_tensor(
                        t_sb[:, h, q0 : q0 + QH], u_sb, rb, ALU.mult,
                    )

                # after each half: queue/run its output projection
                if half == 0:
                    for ti in range(8):
                        for o, w in ((0, 512), (512, 256)):
                            aux_q.append(("outproj", ti, o, w, f"op{ti}o{o}"))
        # flush any aux leftovers before the pool closes
        emit_aux(len(aux_q))

    # ---------- tail: half-1 output projection with a wide pool ----------
    with tc.tile_pool(name="fop", bufs=4, space="PSUM") as fop:
        # keep the PE clock warm while the last normalize chain drains
        wjk = fop.tile([128, 512], F.float32, tag="wjk", name="wjk")
        for _ in range(10):
            nc.tensor.matmul(wjk[0:1, :], lhsT=ones_sb, rhs=wo_sb[:, 0, 0:512],
                             start=True, stop=True, skip_group_check=True)
        for i, ti in enumerate(range(8, 16)):
            for o, w in ((0, 512), (512, 256)):
                cpeng = nc.scalar if (i + (o > 0)) % 2 == 0 else nc.vector
                dmaeng = nc.gpsimd if ti % 2 else nc.sync
                _outproj_chunk(nc, fop, "fo", t_sb, wo_sb, osb, out, ti, o, w,
                               cpeng, dmaeng, f"tp{ti}o{o}")

    if taps:
        nc.sync.dma_start(out=taps["q"][:, :, :], in_=q_sb)
        nc.sync.dma_start(out=taps["k"][:, :, :], in_=k_sb)
        nc.sync.dma_start(out=taps["v"][:, :, :, :], in_=v_sb)
        nc.sync.dma_start(out=taps["t"][:, :, :], in_=t_sb)
        nc.sync.dma_start(out=taps["r"][:, :], in_=r_dram[:, :])


def build_bass(debug_taps=False):
    nc = bacc_mod.Bacc(None)
    xt = nc.dram_tensor("xt", [C, N], F.bfloat16, kind="ExternalInput")
    wq = nc.dram_tensor("wq", [C, HPC * 128], F.bfloat16, kind="ExternalInput")
    wk = nc.dram_tensor("wk", [C, HPC * 128], F.bfloat16, kind="ExternalInput")
    wv = nc.dram_tensor("wv", [C, HPC * D], F.bfloat16, kind="ExternalInput")
    wo = nc.dram_tensor("wo", [2 * HPC * D, C], F.bfloat16, kind="ExternalInput")
    out = nc.dram_tensor("out", [N, C], F.bfloat16, kind="ExternalOutput")
    taps = None
    if debug_taps:
        taps = {
            "q": nc.dram_tensor("tap_q", [128, HPC, N], F.bfloat16, kind="ExternalOutput"),
            "k": nc.dram_tensor("tap_k", [128, HPC, N], F.bfloat16, kind="ExternalOutput"),
            "v": nc.dram_tensor("tap_v", [128, NT, HPC, D], F.bfloat16, kind="ExternalOutput"),
            "t": nc.dram_tensor("tap_t", [128, HPC, N], F.bfloat16, kind="ExternalOutput"),
            "r": nc.dram_tensor("tap_r", [HPC * 2 * 2, QH], F.bfloat16, kind="ExternalOutput"),
        }
    with TileContext(nc) as tc:
        with ExitStack() as ctx:
            _body(nc, tc, ctx, xt, wq, wk, wv, wo, out, taps=taps)
    nc.compile()
    return nc


_NC = None


def _get_nc():
    global _NC
    if _NC is None:
        _NC = build_bass()
    return _NC


def _prep_core(core, x, Wq, Wk, Wv, Wo, lam):
    b = core // 4
    heads = [(core % 4) * HPC + i for i in range(HPC)]
    sc = 1.0 / np.sqrt(D)
    xt = np.ascontiguousarray(x[b].T).astype(BF16)
    wq = np.empty((C, HPC * 128), np.float32)
    wk = np.empty((C, HPC * 128), np.float32)
    for i, h in enumerate(heads):
        for br in range(2):
            c0 = i * 128 + br * 64
            wq[:, c0 : c0 + 64] = Wq[:, br * C + h * D : br * C + (h + 1) * D] * sc
            wk[:, c0 : c0 + 64] = Wk[:, br * C + h * D : br * C + (h + 1) * D]
    wv = np.concatenate([Wv[:, h * D : (h + 1) * D] for h in heads], axis=1)
    # wo: first 192 rows = plain Wo rows (branch0 / partitions 0:64 dup),
    # next 192 rows = -lambda_h * Wo rows (branch1 / partitions 64:128 dup)
    wo0 = np.concatenate([Wo[h * D : (h + 1) * D, :] for h in heads], axis=0)
    wo1 = np.concatenate(
        [-lam[h] * Wo[h * D : (h + 1) * D, :] for h in heads], axis=0
    )
    wo = np.concatenate([wo0, wo1], axis=0)
    return dict(
        xt=xt,
        wq=wq.astype(BF16),
        wk=wk.astype(BF16),
        wv=wv.astype(BF16),
        wo=wo.astype(BF16),
    )


def kernel(x, Wq, Wk, Wv, lambda_p, Wo, bo, _trace=False, _tmpdir=None, _taps=False):
    x = np.asarray(x, np.float32)
    lam = np.exp(np.asarray(lambda_p, np.float32).reshape(H))
    in_maps = [
        _prep_core(core, x, np.asarray(Wq, np.float32), np.asarray(Wk, np.float32),
                   np.asarray(Wv, np.float32), np.asarray(Wo, np.float32), lam)
        for core in range(NCORES)
    ]
    global _NC
    if _taps and _NC is None:
        _NC = build_bass(debug_taps=True)
    nc = _get_nc()
    res = run_bass_kernel_spmd(
        nc, in_maps, list(range(NCORES)), trace=_trace, tmpdir=_tmpdir
    )
    outf = np.zeros((B, N, C), np.float32)
    for core in range(NCORES):
        outf[core // 4] += np.asarray(res.results[core]["out"], np.float32)
    outf += np.asarray(bo, np.float32)[None, None, :]
    if _trace:
        kernel.last_exec_time_ns = res.exec_time_ns
    kernel.last_results = res.results
    return outf
